# revision 29
# baseline (speedup 1.0000x reference)
"""Trainium2 Bass kernel for the Tucker-factorized (TLE) multi-head attention.

Strategy
--------
Data-parallel over batch: 16 batches / 8 cores = 2 batches per core; every
core runs the full per-batch pipeline (no collectives needed).

Host-side prep: the three per-mode factor matrices of each projection are
folded into one dense 768x768 Kronecker matrix.  Rows (for q/k/v) are
permuted to *head-major* order (h1,h2,h3,x,y,z) so each of the 12 heads
occupies a contiguous 64-partition block -- two heads per 128-partition
chunk.  The softmax scale 1/8 is folded into the q matrix/bias.  The o
matrix gets the inverse permutation on its columns.

Device pipeline, per batch (everything feature-major: features on SBUF
partitions, tokens on the free axis):
  1. DMA x [600,768] token-major, PE-transpose to xT [768,600].
  2. q/k/vT = W @ xT + b      (dense matmuls, 6x6 chunk grid, bias on DVE)
  3. per head: S^T = K^T-chunksT @ Q^T (2 heads packed in the PE array via
     row tiling), exp on ScalarE (no max subtraction -- |S|<~8 so exp is
     safe), PV via lhsT=[V|1] (the ones column yields the softmax sums for
     free), normalize with a fast DVE reciprocal + DMA partition-broadcast.
  4. outT = Wo @ yT + bo, PE-transpose back to token-major, DMA out.

Matmuls run in float32r (reduced-precision fp32, 1 cycle/row vs 4 for
fp32).  float32r operand tensors must be written by their producers with a
float32r output dtype (hardware rounds on write), so the tiles feeding
matmuls are allocated as float32r.
"""

import numpy as np

import concourse.bass as bass
import concourse.tile as tile
from concourse import bacc, mybir
from concourse.bass_utils import run_bass_kernel_spmd
from concourse.masks import make_identity

# ---------------------------------------------------------------- constants
N_CORES = 8
B = 16
BPC = B // N_CORES          # batches per core
P1, P2 = 25, 24
S = P1 * P2                 # 600 tokens
F = 768                     # flattened feature dim
FC = 6                      # feature chunks of 128
ST = 120                    # token tile
NS = S // ST                # 5 token tiles
NH = 300                    # half of the token axis (psum-bank sized)
H1, H2, H3 = 2, 2, 3
XD = YD = ZD = 4
NHEADS = H1 * H2 * H3       # 12
HD = 64
F32 = mybir.dt.float32

USE_F32R = True
OP_DT = mybir.dt.float32r if USE_F32R else F32

import os
KDEBUG = os.environ.get("KDEBUG") == "1"


# ---------------------------------------------------------------- device IR
def _build_nc():
    nc = bacc.Bacc("TRN2", target_bir_lowering=False, debug=False)
    xr = nc.declare_dram_parameter("x", [BPC, S, F], F32, isOutput=False)
    ws = [nc.declare_dram_parameter(f"w{m}", [F, F], F32, isOutput=False)
          for m in range(4)]
    bs = [nc.declare_dram_parameter(f"b{m}", [F], F32, isOutput=False)
          for m in range(4)]
    outr = nc.declare_dram_parameter("out", [BPC, S, F], F32, isOutput=True)
    dbg = {}
    if KDEBUG:
        for nm in ("xT", "qT", "kT", "vT", "yT", "outT"):
            dbg[nm] = nc.declare_dram_parameter(f"dbg_{nm}", [128, FC, S], F32,
                                                isOutput=True)
        dbg["pp"] = nc.declare_dram_parameter("dbg_pp", [128, 2, NS, NH], F32,
                                              isOutput=True)
        dbg["vn"] = nc.declare_dram_parameter("dbg_vn", [128, NS, 2, HD + 1], F32,
                                              isOutput=True)

    def _dump(nm, ap):
        if KDEEP := KDEBUG:
            if ap.dtype != F32:
                ap = ap.bitcast(F32)
            nc.sync.dma_start(out=dbg[nm][:], in_=ap)

    with tile.TileContext(nc) as tc:
        from contextlib import ExitStack
        with ExitStack() as ctx:
            const = ctx.enter_context(tc.tile_pool(name="const", bufs=1))
            big = ctx.enter_context(tc.tile_pool(name="big", bufs=1))
            qkvp = ctx.enter_context(tc.tile_pool(name="qkvp", bufs=3))
            stage = ctx.enter_context(tc.tile_pool(name="stage", bufs=2))
            vnp = ctx.enter_context(tc.tile_pool(name="vnp", bufs=2))
            ppool = ctx.enter_context(tc.tile_pool(name="ppool", bufs=2))
            recp = ctx.enter_context(tc.tile_pool(name="recp", bufs=2))
            # PSUM pools -- total bank budget is 8:
            ptr = ctx.enter_context(tc.tile_pool(name="ptr", bufs=1, space="PSUM"))
            pj = ctx.enter_context(tc.tile_pool(name="pj", bufs=2, space="PSUM"))
            ps = ctx.enter_context(tc.tile_pool(name="ps", bufs=3, space="PSUM"))
            py = ctx.enter_context(tc.tile_pool(name="py", bufs=2, space="PSUM"))

            ident = const.tile([128, 128], F32, tag="ident")
            make_identity(nc, ident[:])
            if USE_F32R:
                ident_r = const.tile([128, 128], OP_DT, tag="identr")
                nc.vector.tensor_copy(ident_r[:], ident[:])
            else:
                ident_r = ident
            ones_f = const.tile([128, 1], F32, tag="ones_f")
            nc.gpsimd.memset(ones_f[:], 1.0)
            ones_r = const.tile([128, 1], OP_DT, tag="ones_r")
            nc.vector.tensor_copy(ones_r[:], ones_f[:])

            # weights [128, 6(fi-chunk), 768(fo)] and biases [128, 6(chunk)].
            # Raw fp32 bits DMA straight into f32r tiles (HW matmul treats
            # them identically to pre-rounded values -- probed).  Loaded in
            # per-head-pair column slices, q/k/v slices first, on two DMA
            # queues, so pair-0 attention starts ~3us in instead of waiting
            # 55us for all 9.4MB of weights.
            wsb = []
            bsb = []
            for m in range(4):
                w = const.tile([128, FC, F], OP_DT, tag=f"w{m}")
                wsb.append(w)
                b = const.tile([128, FC], F32, tag=f"b{m}")
                nc.gpsimd.dma_start(out=b[:], in_=bs[m].rearrange("(c p) -> p c", p=128))
                bsb.append(b)
            dma_engs = [nc.sync, nc.gpsimd]
            qcount = 0
            for co in range(FC):
                for m in range(3):
                    eng = dma_engs[qcount % 2]
                    qcount += 1
                    eng.dma_start(
                        out=wsb[m][:, :, co * 128:(co + 1) * 128],
                        in_=ws[m].rearrange("(c p) f -> p c f", p=128)
                        [:, :, co * 128:(co + 1) * 128].bitcast(OP_DT))
            for co in range(FC):
                eng = dma_engs[qcount % 2]
                qcount += 1
                eng.dma_start(
                    out=wsb[3][:, :, co * 128:(co + 1) * 128],
                    in_=ws[3].rearrange("(c p) f -> p c f", p=128)
                    [:, :, co * 128:(co + 1) * 128].bitcast(OP_DT))

            for b in range(BPC):
                # ---- load x + transpose to feature-major -------------------
                xT = big.tile([128, FC, S], OP_DT, tag="xyT")
                for st in range(NS):
                    xn = stage.tile([128, F], F32, tag="xn")
                    nc.scalar.dma_start(out=xn[:ST, :], in_=xr[b, st * ST:(st + 1) * ST, :])
                    for c0 in range(0, FC, 4):
                        nch = min(4, FC - c0)
                        pt = ptr.tile([128, 512], F32, tag="ptr")
                        for t in range(nch):
                            nc.tensor.transpose(
                                pt[:, t * 128:t * 128 + ST],
                                xn[:ST, (c0 + t) * 128:(c0 + t + 1) * 128],
                                ident[:ST, :ST])
                        nc.vector.tensor_copy(
                            xT[:, c0:c0 + nch, st * ST:(st + 1) * ST],
                            pt[:, :nch * 128].rearrange("p (t s) -> p t s", t=nch)[:, :, :ST])

                if b == 0:
                    _dump("xT", xT[:])

                # ---- fused per-pair projections + attention ---------------
                # chunk co of head-major q/k/v == head pair co, so project
                # one chunk of q,k,v then immediately run that pair's
                # attention; projection matmuls of pair n+1 (PE) overlap the
                # exp of pair n (ScalarE).
                qT = qkvp.tile([128, FC, S], OP_DT, tag="qkvT")
                kT = qkvp.tile([128, FC, S], OP_DT, tag="qkvT")
                vT = qkvp.tile([128, FC, S], OP_DT, tag="qkvT")
                qkvT = [qT, kT, vT]
                yT = big.tile([128, FC, S], OP_DT, tag="xyT")
                for hp in range(FC):
                    for m in range(3):
                        dst = qkvT[m]
                        acc0 = pj.tile([128, 512], F32, tag="pj")
                        acc1 = pj.tile([128, 512], F32, tag="pj")
                        accs = (acc0, acc1)
                        for ci in range(FC):
                            for h in range(2):
                                nc.tensor.matmul(
                                    accs[h][:, :NH],
                                    wsb[m][:, ci, hp * 128:(hp + 1) * 128],
                                    xT[:, ci, h * NH:(h + 1) * NH],
                                    start=(ci == 0), stop=(ci == FC - 1))
                        for h in range(2):
                            nc.vector.tensor_scalar_add(
                                dst[:, hp, h * NH:(h + 1) * NH],
                                in0=accs[h][:, :NH], scalar1=bsb[m][:, hp:hp + 1])

                    # V back to token-major, with an appended ones column
                    vn = vnp.tile([128, NS, 2, HD + 1], OP_DT, tag="vn")
                    nc.vector.tensor_copy(
                        vn[:ST, :, :, HD:HD + 1],
                        ones_r[:ST, 0:1].to_broadcast((ST, NS, 2, 1)))
                    for t0 in range(0, NS, 4):
                        nch = min(4, NS - t0)
                        pt = ptr.tile([128, 512], F32, tag="ptr")
                        for t in range(nch):
                            nc.tensor.transpose(
                                pt[:ST, t * 128:(t + 1) * 128].bitcast(OP_DT),
                                vT[:, hp, (t0 + t) * ST:(t0 + t + 1) * ST],
                                ident_r[:, :])
                        nc.vector.tensor_copy(
                            vn[:ST, t0:t0 + nch, :, 0:HD],
                            pt[:ST, :nch * 128].rearrange("p (t g d) -> p t g d", t=nch, g=2))
                    if b == 0 and hp == 0:
                        _dump("vn", vn[:])
                    for sh in range(2):
                        pp = ppool.tile([128, 2, NS, NH], OP_DT, tag="pp")
                        for g in range(2):
                            r0 = g * HD
                            for t5 in range(NS):
                                st_ps = ps.tile([128, 512], F32, tag="ps")
                                nc.tensor.matmul(
                                    st_ps[:ST, :NH],
                                    kT[r0:r0 + HD, hp, t5 * ST:(t5 + 1) * ST],
                                    qT[r0:r0 + HD, hp, sh * NH:(sh + 1) * NH],
                                    start=True, stop=True)
                                nc.scalar.activation(
                                    pp[:ST, g, t5, :], st_ps[:ST, :NH],
                                    func=mybir.ActivationFunctionType.Exp)
                            if b == 0 and hp == 0 and sh == 0 and g == 1:
                                _dump("pp", pp[:])
                            acc = py.tile([HD + 1, 512], F32, tag="py")
                            for t5 in range(NS):
                                nc.tensor.matmul(
                                    acc[:HD + 1, :NH],
                                    vn[:ST, t5, g, :],
                                    pp[:ST, g, t5, :],
                                    start=(t5 == 0), stop=(t5 == NS - 1))
                            srow = recp.tile([1, NH], F32, tag="srow")
                            nc.vector.tensor_copy(srow[:, :], acc[HD:HD + 1, :NH])
                            rec = recp.tile([1, NH], F32, tag="rec")
                            nc.vector.reciprocal_approx_fast(rec[:, :], srow[:, :])
                            rb = recp.tile([HD, NH], F32, tag="rb")
                            nc.gpsimd.partition_broadcast(rb[:, :], rec[0:1, :])
                            nc.vector.tensor_mul(
                                yT[r0:r0 + HD, hp, sh * NH:(sh + 1) * NH],
                                acc[:HD, :NH], rb[:, :])
                if b == 0:
                    _dump("qT", qT[:])
                    _dump("kT", kT[:])
                    _dump("vT", vT[:])
                    _dump("yT", yT[:])

                # ---- output projection ------------------------------------
                outT = big.tile([128, FC, S], F32, tag="outT")
                for co in range(FC):
                    acc0 = pj.tile([128, 512], F32, tag="pj")
                    acc1 = pj.tile([128, 512], F32, tag="pj")
                    accs = (acc0, acc1)
                    for ci in range(FC):
                        for h in range(2):
                            nc.tensor.matmul(
                                accs[h][:, :NH],
                                wsb[3][:, ci, co * 128:(co + 1) * 128],
                                yT[:, ci, h * NH:(h + 1) * NH],
                                start=(ci == 0), stop=(ci == FC - 1))
                    for h in range(2):
                        nc.vector.tensor_scalar_add(
                            outT[:, co, h * NH:(h + 1) * NH],
                            in0=accs[h][:, :NH], scalar1=bsb[3][:, co:co + 1])

                if b == 0:
                    _dump("outT", outT[:])

                # ---- back to token-major + store --------------------------
                for st in range(NS):
                    on = stage.tile([128, F], F32, tag="on")
                    for c0 in range(0, FC, 4):
                        nch = min(4, FC - c0)
                        pt = ptr.tile([128, 512], F32, tag="ptr")
                        for t in range(nch):
                            nc.tensor.transpose(
                                pt[:ST, t * 128:(t + 1) * 128],
                                outT[:, c0 + t, st * ST:(st + 1) * ST],
                                ident[:, :])
                        nc.vector.tensor_copy(
                            on[:ST, c0 * 128:(c0 + nch) * 128], pt[:ST, :nch * 128])
                    nc.sync.dma_start(out=outr[b, st * ST:(st + 1) * ST, :], in_=on[:ST, :])

    nc.finalize()
    return nc


_NC_CACHE = {}


def _get_nc():
    if "nc" not in _NC_CACHE:
        _NC_CACHE["nc"] = _build_nc()
    return _NC_CACHE["nc"]


# ------------------------------------------------------------- host wrapper
def _head_major_perm():
    perm = np.empty(F, dtype=np.int64)
    i = 0
    for h1 in range(H1):
        for h2 in range(H2):
            for h3 in range(H3):
                for x in range(XD):
                    for y in range(YD):
                        for z in range(ZD):
                            a = x * H1 + h1
                            bb = y * H2 + h2
                            cc = z * H3 + h3
                            perm[i] = a * 96 + bb * 12 + cc
                            i += 1
    return perm


def _prep_inputs(inputs):
    perm = _head_major_perm()
    scale = float(HD) ** -0.5

    def kron3(w1, w2, w3):
        return np.kron(w1, np.kron(w2, w3)).astype(np.float32)

    mats = {}
    mats["w0"] = np.ascontiguousarray(
        (kron3(inputs["Wq1"], inputs["Wq2"], inputs["Wq3"])[perm, :] * scale).T)
    mats["b0"] = np.ascontiguousarray(
        inputs["bq"].reshape(F)[perm] * scale).astype(np.float32)
    mats["w1"] = np.ascontiguousarray(
        kron3(inputs["Wk1"], inputs["Wk2"], inputs["Wk3"])[perm, :].T)
    mats["b1"] = np.ascontiguousarray(inputs["bk"].reshape(F)[perm]).astype(np.float32)
    mats["w2"] = np.ascontiguousarray(
        kron3(inputs["Wv1"], inputs["Wv2"], inputs["Wv3"])[perm, :].T)
    mats["b2"] = np.ascontiguousarray(inputs["bv"].reshape(F)[perm]).astype(np.float32)
    mats["w3"] = np.ascontiguousarray(
        kron3(inputs["Wo1"], inputs["Wo2"], inputs["Wo3"])[:, perm].T)
    mats["b3"] = np.ascontiguousarray(inputs["bo"].reshape(F)).astype(np.float32)
    return mats


def _make_in_maps(inputs):
    mats = _prep_inputs(inputs)
    x = np.ascontiguousarray(np.asarray(inputs["x"], dtype=np.float32).reshape(B, S, F))
    in_maps = []
    for c in range(N_CORES):
        m = {"x": np.ascontiguousarray(x[c * BPC:(c + 1) * BPC])}
        m.update(mats)
        in_maps.append(m)
    return in_maps


def kernel(**inputs) -> np.ndarray:
    nc = _get_nc()
    in_maps = _make_in_maps(inputs)
    res = run_bass_kernel_spmd(nc, in_maps, core_ids=list(range(N_CORES)))
    out = np.concatenate([res.results[c]["out"] for c in range(N_CORES)], axis=0)
    return out.reshape(B, P1, P2, 8, 8, 12).astype(np.float32)


def run_traced(inputs, **kw):
    """test.py helper: returns (output, BassKernelResults) with trace."""
    nc = _get_nc()
    in_maps = _make_in_maps(inputs)
    res = run_bass_kernel_spmd(nc, in_maps, core_ids=list(range(N_CORES)), **kw)
    out = np.concatenate([res.results[c]["out"] for c in range(N_CORES)], axis=0)
    return out.reshape(B, P1, P2, 8, 8, 12).astype(np.float32), res


# revision 30
# speedup vs baseline: 1.0088x; 1.0088x over previous
"""Trainium2 Bass kernel for the Tucker-factorized (TLE) multi-head attention.

Strategy
--------
Data-parallel over batch: 16 batches / 8 cores = 2 batches per core; every
core runs the full per-batch pipeline (no collectives needed).

Host-side prep: the three per-mode factor matrices of each projection are
folded into one dense 768x768 Kronecker matrix.  Rows (for q/k/v) are
permuted to *head-major* order (h1,h2,h3,x,y,z) so each of the 12 heads
occupies a contiguous 64-partition block -- two heads per 128-partition
chunk.  The softmax scale 1/8 is folded into the q matrix/bias.  The o
matrix gets the inverse permutation on its columns.

Device pipeline, per batch (everything feature-major: features on SBUF
partitions, tokens on the free axis):
  1. DMA x [600,768] token-major, PE-transpose to xT [768,600].
  2. q/k/vT = W @ xT + b      (dense matmuls, 6x6 chunk grid, bias on DVE)
  3. per head: S^T = K^T-chunksT @ Q^T (2 heads packed in the PE array via
     row tiling), exp on ScalarE (no max subtraction -- |S|<~8 so exp is
     safe), PV via lhsT=[V|1] (the ones column yields the softmax sums for
     free), normalize with a fast DVE reciprocal + DMA partition-broadcast.
  4. outT = Wo @ yT + bo, PE-transpose back to token-major, DMA out.

Matmuls run in float32r (reduced-precision fp32, 1 cycle/row vs 4 for
fp32).  float32r operand tensors must be written by their producers with a
float32r output dtype (hardware rounds on write), so the tiles feeding
matmuls are allocated as float32r.
"""

import numpy as np

import concourse.bass as bass
import concourse.tile as tile
from concourse import bacc, mybir
from concourse.bass_utils import run_bass_kernel_spmd
from concourse.masks import make_identity

# ---------------------------------------------------------------- constants
N_CORES = 8
B = 16
BPC = B // N_CORES          # batches per core
P1, P2 = 25, 24
S = P1 * P2                 # 600 tokens
F = 768                     # flattened feature dim
FC = 6                      # feature chunks of 128
ST = 120                    # token tile
NS = S // ST                # 5 token tiles
NH = 300                    # half of the token axis (psum-bank sized)
H1, H2, H3 = 2, 2, 3
XD = YD = ZD = 4
NHEADS = H1 * H2 * H3       # 12
HD = 64
F32 = mybir.dt.float32

USE_F32R = True
OP_DT = mybir.dt.float32r if USE_F32R else F32

import os
KDEBUG = os.environ.get("KDEBUG") == "1"


# ---------------------------------------------------------------- device IR
def _build_nc():
    nc = bacc.Bacc("TRN2", target_bir_lowering=False, debug=False)
    xr = nc.declare_dram_parameter("x", [BPC, S, F], F32, isOutput=False)
    ws = [nc.declare_dram_parameter(f"w{m}", [F, F], F32, isOutput=False)
          for m in range(4)]
    bs = [nc.declare_dram_parameter(f"b{m}", [F], F32, isOutput=False)
          for m in range(4)]
    outr = nc.declare_dram_parameter("out", [BPC, S, F], F32, isOutput=True)
    dbg = {}
    if KDEBUG:
        for nm in ("xT", "qT", "kT", "vT", "yT", "outT"):
            dbg[nm] = nc.declare_dram_parameter(f"dbg_{nm}", [128, FC, S], F32,
                                                isOutput=True)
        dbg["pp"] = nc.declare_dram_parameter("dbg_pp", [128, 2, NS, NH], F32,
                                              isOutput=True)
        dbg["vn"] = nc.declare_dram_parameter("dbg_vn", [128, NS, 2, HD + 1], F32,
                                              isOutput=True)

    def _dump(nm, ap):
        if KDEEP := KDEBUG:
            if ap.dtype != F32:
                ap = ap.bitcast(F32)
            nc.sync.dma_start(out=dbg[nm][:], in_=ap)

    with tile.TileContext(nc) as tc:
        from contextlib import ExitStack
        with ExitStack() as ctx:
            const = ctx.enter_context(tc.tile_pool(name="const", bufs=1))
            big = ctx.enter_context(tc.tile_pool(name="big", bufs=1))
            qkvp = ctx.enter_context(tc.tile_pool(name="qkvp", bufs=3))
            stage = ctx.enter_context(tc.tile_pool(name="stage", bufs=2))
            vnp = ctx.enter_context(tc.tile_pool(name="vnp", bufs=2))
            ppool = ctx.enter_context(tc.tile_pool(name="ppool", bufs=2))
            recp = ctx.enter_context(tc.tile_pool(name="recp", bufs=2))
            # PSUM pools -- total bank budget is 8:
            ptr = ctx.enter_context(tc.tile_pool(name="ptr", bufs=1, space="PSUM"))
            pj = ctx.enter_context(tc.tile_pool(name="pj", bufs=3, space="PSUM"))
            ps = ctx.enter_context(tc.tile_pool(name="ps", bufs=2, space="PSUM"))
            py = ctx.enter_context(tc.tile_pool(name="py", bufs=2, space="PSUM"))

            ident = const.tile([128, 128], F32, tag="ident")
            make_identity(nc, ident[:])
            if USE_F32R:
                ident_r = const.tile([128, 128], OP_DT, tag="identr")
                nc.vector.tensor_copy(ident_r[:], ident[:])
            else:
                ident_r = ident
            ones_f = const.tile([128, 1], F32, tag="ones_f")
            nc.gpsimd.memset(ones_f[:], 1.0)
            ones_r = const.tile([128, 1], OP_DT, tag="ones_r")
            nc.vector.tensor_copy(ones_r[:], ones_f[:])

            # weights [128, 6(fi-chunk), 768(fo)] and biases [128, 6(chunk)].
            # Raw fp32 bits DMA straight into f32r tiles (HW matmul treats
            # them identically to pre-rounded values -- probed).  Loaded in
            # per-head-pair column slices, q/k/v slices first, on two DMA
            # queues, so pair-0 attention starts ~3us in instead of waiting
            # 55us for all 9.4MB of weights.
            wsb = []
            bsb = []
            for m in range(4):
                w = const.tile([128, FC, F], OP_DT, tag=f"w{m}")
                wsb.append(w)
                b = const.tile([128, FC], F32, tag=f"b{m}")
                nc.gpsimd.dma_start(out=b[:], in_=bs[m].rearrange("(c p) -> p c", p=128))
                bsb.append(b)
            dma_engs = [nc.sync, nc.gpsimd]
            qcount = 0
            for co in range(FC):
                for m in range(3):
                    eng = dma_engs[qcount % 2]
                    qcount += 1
                    eng.dma_start(
                        out=wsb[m][:, :, co * 128:(co + 1) * 128],
                        in_=ws[m].rearrange("(c p) f -> p c f", p=128)
                        [:, :, co * 128:(co + 1) * 128].bitcast(OP_DT))
            for co in range(FC):
                eng = dma_engs[qcount % 2]
                qcount += 1
                eng.dma_start(
                    out=wsb[3][:, :, co * 128:(co + 1) * 128],
                    in_=ws[3].rearrange("(c p) f -> p c f", p=128)
                    [:, :, co * 128:(co + 1) * 128].bitcast(OP_DT))

            for b in range(BPC):
                # ---- load x + transpose to feature-major -------------------
                xT = big.tile([128, FC, S], OP_DT, tag="xyT")
                for st in range(NS):
                    xn = stage.tile([128, F], F32, tag="xn")
                    nc.scalar.dma_start(out=xn[:ST, :], in_=xr[b, st * ST:(st + 1) * ST, :])
                    for c0 in range(0, FC, 4):
                        nch = min(4, FC - c0)
                        pt = ptr.tile([128, 512], F32, tag="ptr")
                        for t in range(nch):
                            nc.tensor.transpose(
                                pt[:, t * 128:t * 128 + ST],
                                xn[:ST, (c0 + t) * 128:(c0 + t + 1) * 128],
                                ident[:ST, :ST])
                        nc.vector.tensor_copy(
                            xT[:, c0:c0 + nch, st * ST:(st + 1) * ST],
                            pt[:, :nch * 128].rearrange("p (t s) -> p t s", t=nch)[:, :, :ST])

                if b == 0:
                    _dump("xT", xT[:])

                # ---- fused per-pair projections + attention ---------------
                # chunk co of head-major q/k/v == head pair co, so project
                # one chunk of q,k,v then immediately run that pair's
                # attention; projection matmuls of pair n+1 (PE) overlap the
                # exp of pair n (ScalarE).
                qT = qkvp.tile([128, FC, S], OP_DT, tag="qkvT")
                kT = qkvp.tile([128, FC, S], OP_DT, tag="qkvT")
                vT = qkvp.tile([128, FC, S], OP_DT, tag="qkvT")
                qkvT = [qT, kT, vT]
                yT = big.tile([128, FC, S], OP_DT, tag="xyT")
                for hp in range(FC):
                    for m in range(3):
                        dst = qkvT[m]
                        acc0 = pj.tile([128, 512], F32, tag="pj")
                        acc1 = pj.tile([128, 512], F32, tag="pj")
                        accs = (acc0, acc1)
                        for ci in range(FC):
                            for h in range(2):
                                nc.tensor.matmul(
                                    accs[h][:, :NH],
                                    wsb[m][:, ci, hp * 128:(hp + 1) * 128],
                                    xT[:, ci, h * NH:(h + 1) * NH],
                                    start=(ci == 0), stop=(ci == FC - 1))
                        for h in range(2):
                            nc.vector.tensor_scalar_add(
                                dst[:, hp, h * NH:(h + 1) * NH],
                                in0=accs[h][:, :NH], scalar1=bsb[m][:, hp:hp + 1])

                    # V back to token-major, with an appended ones column
                    vn = vnp.tile([128, NS, 2, HD + 1], OP_DT, tag="vn")
                    nc.vector.tensor_copy(
                        vn[:ST, :, :, HD:HD + 1],
                        ones_r[:ST, 0:1].to_broadcast((ST, NS, 2, 1)))
                    for t0 in range(0, NS, 4):
                        nch = min(4, NS - t0)
                        pt = ptr.tile([128, 512], F32, tag="ptr")
                        for t in range(nch):
                            nc.tensor.transpose(
                                pt[:ST, t * 128:(t + 1) * 128].bitcast(OP_DT),
                                vT[:, hp, (t0 + t) * ST:(t0 + t + 1) * ST],
                                ident_r[:, :])
                        nc.vector.tensor_copy(
                            vn[:ST, t0:t0 + nch, :, 0:HD],
                            pt[:ST, :nch * 128].rearrange("p (t g d) -> p t g d", t=nch, g=2))
                    if b == 0 and hp == 0:
                        _dump("vn", vn[:])
                    for sh in range(2):
                        pp = ppool.tile([128, 2, NS, NH], OP_DT, tag="pp")
                        for g in range(2):
                            r0 = g * HD
                            for t5 in range(NS):
                                st_ps = ps.tile([128, 512], F32, tag="ps")
                                nc.tensor.matmul(
                                    st_ps[:ST, :NH],
                                    kT[r0:r0 + HD, hp, t5 * ST:(t5 + 1) * ST],
                                    qT[r0:r0 + HD, hp, sh * NH:(sh + 1) * NH],
                                    start=True, stop=True)
                                nc.scalar.activation(
                                    pp[:ST, g, t5, :], st_ps[:ST, :NH],
                                    func=mybir.ActivationFunctionType.Exp)
                            if b == 0 and hp == 0 and sh == 0 and g == 1:
                                _dump("pp", pp[:])
                            acc = py.tile([HD + 1, 512], F32, tag="py")
                            for t5 in range(NS):
                                nc.tensor.matmul(
                                    acc[:HD + 1, :NH],
                                    vn[:ST, t5, g, :],
                                    pp[:ST, g, t5, :],
                                    start=(t5 == 0), stop=(t5 == NS - 1))
                            srow = recp.tile([1, NH], F32, tag="srow")
                            nc.vector.tensor_copy(srow[:, :], acc[HD:HD + 1, :NH])
                            rec = recp.tile([1, NH], F32, tag="rec")
                            nc.vector.reciprocal_approx_fast(rec[:, :], srow[:, :])
                            rb = recp.tile([HD, NH], F32, tag="rb")
                            nc.gpsimd.partition_broadcast(rb[:, :], rec[0:1, :])
                            nc.vector.tensor_mul(
                                yT[r0:r0 + HD, hp, sh * NH:(sh + 1) * NH],
                                acc[:HD, :NH], rb[:, :])
                if b == 0:
                    _dump("qT", qT[:])
                    _dump("kT", kT[:])
                    _dump("vT", vT[:])
                    _dump("yT", yT[:])

                # ---- output projection ------------------------------------
                outT = big.tile([128, FC, S], F32, tag="outT")
                for co in range(FC):
                    acc0 = pj.tile([128, 512], F32, tag="pj")
                    acc1 = pj.tile([128, 512], F32, tag="pj")
                    accs = (acc0, acc1)
                    for ci in range(FC):
                        for h in range(2):
                            nc.tensor.matmul(
                                accs[h][:, :NH],
                                wsb[3][:, ci, co * 128:(co + 1) * 128],
                                yT[:, ci, h * NH:(h + 1) * NH],
                                start=(ci == 0), stop=(ci == FC - 1))
                    for h in range(2):
                        nc.vector.tensor_scalar_add(
                            outT[:, co, h * NH:(h + 1) * NH],
                            in0=accs[h][:, :NH], scalar1=bsb[3][:, co:co + 1])

                if b == 0:
                    _dump("outT", outT[:])

                # ---- back to token-major + store --------------------------
                for st in range(NS):
                    on = stage.tile([128, F], F32, tag="on")
                    for c0 in range(0, FC, 4):
                        nch = min(4, FC - c0)
                        pt = ptr.tile([128, 512], F32, tag="ptr")
                        for t in range(nch):
                            nc.tensor.transpose(
                                pt[:ST, t * 128:(t + 1) * 128],
                                outT[:, c0 + t, st * ST:(st + 1) * ST],
                                ident[:, :])
                        nc.vector.tensor_copy(
                            on[:ST, c0 * 128:(c0 + nch) * 128], pt[:ST, :nch * 128])
                    nc.sync.dma_start(out=outr[b, st * ST:(st + 1) * ST, :], in_=on[:ST, :])

    nc.finalize()
    return nc


_NC_CACHE = {}


def _get_nc():
    if "nc" not in _NC_CACHE:
        _NC_CACHE["nc"] = _build_nc()
    return _NC_CACHE["nc"]


# ------------------------------------------------------------- host wrapper
def _head_major_perm():
    perm = np.empty(F, dtype=np.int64)
    i = 0
    for h1 in range(H1):
        for h2 in range(H2):
            for h3 in range(H3):
                for x in range(XD):
                    for y in range(YD):
                        for z in range(ZD):
                            a = x * H1 + h1
                            bb = y * H2 + h2
                            cc = z * H3 + h3
                            perm[i] = a * 96 + bb * 12 + cc
                            i += 1
    return perm


def _prep_inputs(inputs):
    perm = _head_major_perm()
    scale = float(HD) ** -0.5

    def kron3(w1, w2, w3):
        return np.kron(w1, np.kron(w2, w3)).astype(np.float32)

    mats = {}
    mats["w0"] = np.ascontiguousarray(
        (kron3(inputs["Wq1"], inputs["Wq2"], inputs["Wq3"])[perm, :] * scale).T)
    mats["b0"] = np.ascontiguousarray(
        inputs["bq"].reshape(F)[perm] * scale).astype(np.float32)
    mats["w1"] = np.ascontiguousarray(
        kron3(inputs["Wk1"], inputs["Wk2"], inputs["Wk3"])[perm, :].T)
    mats["b1"] = np.ascontiguousarray(inputs["bk"].reshape(F)[perm]).astype(np.float32)
    mats["w2"] = np.ascontiguousarray(
        kron3(inputs["Wv1"], inputs["Wv2"], inputs["Wv3"])[perm, :].T)
    mats["b2"] = np.ascontiguousarray(inputs["bv"].reshape(F)[perm]).astype(np.float32)
    mats["w3"] = np.ascontiguousarray(
        kron3(inputs["Wo1"], inputs["Wo2"], inputs["Wo3"])[:, perm].T)
    mats["b3"] = np.ascontiguousarray(inputs["bo"].reshape(F)).astype(np.float32)
    return mats


def _make_in_maps(inputs):
    mats = _prep_inputs(inputs)
    x = np.ascontiguousarray(np.asarray(inputs["x"], dtype=np.float32).reshape(B, S, F))
    in_maps = []
    for c in range(N_CORES):
        m = {"x": np.ascontiguousarray(x[c * BPC:(c + 1) * BPC])}
        m.update(mats)
        in_maps.append(m)
    return in_maps


def kernel(**inputs) -> np.ndarray:
    nc = _get_nc()
    in_maps = _make_in_maps(inputs)
    res = run_bass_kernel_spmd(nc, in_maps, core_ids=list(range(N_CORES)))
    out = np.concatenate([res.results[c]["out"] for c in range(N_CORES)], axis=0)
    return out.reshape(B, P1, P2, 8, 8, 12).astype(np.float32)


def run_traced(inputs, **kw):
    """test.py helper: returns (output, BassKernelResults) with trace."""
    nc = _get_nc()
    in_maps = _make_in_maps(inputs)
    res = run_bass_kernel_spmd(nc, in_maps, core_ids=list(range(N_CORES)), **kw)
    out = np.concatenate([res.results[c]["out"] for c in range(N_CORES)], axis=0)
    return out.reshape(B, P1, P2, 8, 8, 12).astype(np.float32), res


# revision 32
# speedup vs baseline: 1.0996x; 1.0900x over previous
"""Trainium2 Bass kernel for the Tucker-factorized (TLE) multi-head attention.

Strategy
--------
Data-parallel over batch: 16 batches / 8 cores = 2 batches per core; every
core runs the full per-batch pipeline (no collectives needed).

Host-side prep: the three per-mode factor matrices of each projection are
folded into one dense 768x768 Kronecker matrix.  Rows (for q/k/v) are
permuted to *head-major* order (h1,h2,h3,x,y,z) so each of the 12 heads
occupies a contiguous 64-partition block -- two heads per 128-partition
chunk.  The softmax scale 1/8 is folded into the q matrix/bias.  The o
matrix gets the inverse permutation on its columns.

Device pipeline, per batch (everything feature-major: features on SBUF
partitions, tokens on the free axis):
  1. DMA x [600,768] token-major, PE-transpose to xT [768,600].
  2. q/k/vT = W @ xT + b      (dense matmuls, 6x6 chunk grid, bias on DVE)
  3. per head: S^T = K^T-chunksT @ Q^T (2 heads packed in the PE array via
     row tiling), exp on ScalarE (no max subtraction -- |S|<~8 so exp is
     safe), PV via lhsT=[V|1] (the ones column yields the softmax sums for
     free), normalize with a fast DVE reciprocal + DMA partition-broadcast.
  4. outT = Wo @ yT + bo, PE-transpose back to token-major, DMA out.

Matmuls run in float32r (reduced-precision fp32, 1 cycle/row vs 4 for
fp32).  float32r operand tensors must be written by their producers with a
float32r output dtype (hardware rounds on write), so the tiles feeding
matmuls are allocated as float32r.
"""

import os

import numpy as np

import concourse.bass as bass
import concourse.tile as tile
from concourse import bacc, mybir
from concourse.bass_utils import run_bass_kernel_spmd
from concourse.masks import make_identity

# ---------------------------------------------------------------- constants
N_CORES = 8
B = 16
BPC = B // N_CORES          # batches per core
P1, P2 = 25, 24
S = P1 * P2                 # 600 tokens
F = 768                     # flattened feature dim
FC = 6                      # feature chunks of 128
ST = 120                    # token tile
NS = S // ST                # 5 token tiles
NH = 300                    # half of the token axis (psum-bank sized)
H1, H2, H3 = 2, 2, 3
XD = YD = ZD = 4
NHEADS = H1 * H2 * H3       # 12
HD = 64
F32 = mybir.dt.float32

USE_F32R = True
OP_DT = mybir.dt.float16 if os.environ.get("KF16") == "1" else (
    mybir.dt.float32r if USE_F32R else F32)
import os as _os
W_DT = OP_DT if OP_DT == mybir.dt.float16 else F32  # DRAM weight dtype

KDEBUG = os.environ.get("KDEBUG") == "1"


# ---------------------------------------------------------------- device IR
def _build_nc():
    nc = bacc.Bacc("TRN2", target_bir_lowering=False, debug=False)
    xr = nc.declare_dram_parameter("x", [BPC, S, F], F32, isOutput=False)
    ws = [nc.declare_dram_parameter(f"w{m}", [F, F], W_DT, isOutput=False)
          for m in range(4)]
    bs = [nc.declare_dram_parameter(f"b{m}", [F], F32, isOutput=False)
          for m in range(4)]
    outr = nc.declare_dram_parameter("out", [BPC, S, F], F32, isOutput=True)
    dbg = {}
    if KDEBUG:
        for nm in ("xT", "qT", "kT", "vT", "yT", "outT"):
            dbg[nm] = nc.declare_dram_parameter(f"dbg_{nm}", [128, FC, S], F32,
                                                isOutput=True)
        dbg["pp"] = nc.declare_dram_parameter("dbg_pp", [128, 2, NS, NH], F32,
                                              isOutput=True)
        dbg["vn"] = nc.declare_dram_parameter("dbg_vn", [128, NS, 2, HD + 1], F32,
                                              isOutput=True)

    def _dump(nm, ap):
        if KDEEP := KDEBUG:
            if ap.dtype != F32:
                ap = ap.bitcast(F32)
            nc.sync.dma_start(out=dbg[nm][:], in_=ap)

    with tile.TileContext(nc) as tc:
        from contextlib import ExitStack
        with ExitStack() as ctx:
            const = ctx.enter_context(tc.tile_pool(name="const", bufs=1))
            big = ctx.enter_context(tc.tile_pool(name="big", bufs=1))
            qkvp = ctx.enter_context(tc.tile_pool(name="qkvp", bufs=3))
            stage = ctx.enter_context(tc.tile_pool(name="stage", bufs=2))
            vnp = ctx.enter_context(tc.tile_pool(name="vnp", bufs=2))
            ppool = ctx.enter_context(tc.tile_pool(name="ppool", bufs=2))
            recp = ctx.enter_context(tc.tile_pool(name="recp", bufs=2))
            # PSUM pools -- total bank budget is 8:
            ptr = ctx.enter_context(tc.tile_pool(name="ptr", bufs=1, space="PSUM"))
            pj = ctx.enter_context(tc.tile_pool(
                name="pj", bufs=2 if OP_DT == mybir.dt.float16 else 3, space="PSUM"))
            ps = ctx.enter_context(tc.tile_pool(name="ps", bufs=2, space="PSUM"))
            py = ctx.enter_context(tc.tile_pool(name="py", bufs=2, space="PSUM"))

            ident = const.tile([128, 128], F32, tag="ident")
            make_identity(nc, ident[:])
            if USE_F32R:
                ident_r = const.tile([128, 128], OP_DT, tag="identr")
                nc.vector.tensor_copy(ident_r[:], ident[:])
            else:
                ident_r = ident
            ones_f = const.tile([128, 1], F32, tag="ones_f")
            nc.gpsimd.memset(ones_f[:], 1.0)
            ones_r = const.tile([128, 1], OP_DT, tag="ones_r")
            nc.vector.tensor_copy(ones_r[:], ones_f[:])

            # weights [128, 6(fi-chunk), 768(fo)] and biases [128, 6(chunk)].
            # Raw fp32 bits DMA straight into f32r tiles (HW matmul treats
            # them identically to pre-rounded values -- probed).  Loaded in
            # per-head-pair column slices, q/k/v slices first, on two DMA
            # queues, so pair-0 attention starts ~3us in instead of waiting
            # 55us for all 9.4MB of weights.
            wsb = []
            bsb = []
            for m in range(4):
                w = const.tile([128, FC, F], OP_DT, tag=f"w{m}")
                wsb.append(w)
                b = const.tile([128, FC], F32, tag=f"b{m}")
                nc.gpsimd.dma_start(out=b[:], in_=bs[m].rearrange("(c p) -> p c", p=128))
                bsb.append(b)
            dma_engs = [nc.sync, nc.gpsimd]
            qcount = 0
            for co in range(FC):
                for m in range(3):
                    eng = dma_engs[qcount % 2]
                    qcount += 1
                    src = ws[m].rearrange("(c p) f -> p c f", p=128)[
                        :, :, co * 128:(co + 1) * 128]
                    if src.dtype != OP_DT:
                        src = src.bitcast(OP_DT)
                    eng.dma_start(out=wsb[m][:, :, co * 128:(co + 1) * 128], in_=src)
            for co in range(FC):
                eng = dma_engs[qcount % 2]
                qcount += 1
                src = ws[3].rearrange("(c p) f -> p c f", p=128)[
                    :, :, co * 128:(co + 1) * 128]
                if src.dtype != OP_DT:
                    src = src.bitcast(OP_DT)
                eng.dma_start(out=wsb[3][:, :, co * 128:(co + 1) * 128], in_=src)

            for b in range(BPC):
                # ---- load x + transpose to feature-major -------------------
                xT = big.tile([128, FC, S], OP_DT, tag="xyT")
                for st in range(NS):
                    xn = stage.tile([128, F], F32, tag="xn")
                    nc.scalar.dma_start(out=xn[:ST, :], in_=xr[b, st * ST:(st + 1) * ST, :])
                    for c0 in range(0, FC, 4):
                        nch = min(4, FC - c0)
                        pt = ptr.tile([128, 512], F32, tag="ptr")
                        for t in range(nch):
                            nc.tensor.transpose(
                                pt[:, t * 128:t * 128 + ST],
                                xn[:ST, (c0 + t) * 128:(c0 + t + 1) * 128],
                                ident[:ST, :ST])
                        nc.vector.tensor_copy(
                            xT[:, c0:c0 + nch, st * ST:(st + 1) * ST],
                            pt[:, :nch * 128].rearrange("p (t s) -> p t s", t=nch)[:, :, :ST])

                if b == 0:
                    _dump("xT", xT[:])

                # ---- fused per-pair projections + attention ---------------
                # chunk co of head-major q/k/v == head pair co, so project
                # one chunk of q,k,v then immediately run that pair's
                # attention; projection matmuls of pair n+1 (PE) overlap the
                # exp of pair n (ScalarE).
                qT = qkvp.tile([128, FC, S], OP_DT, tag="qkvT")
                kT = qkvp.tile([128, FC, S], OP_DT, tag="qkvT")
                vT = qkvp.tile([128, FC, S], OP_DT, tag="qkvT")
                qkvT = [qT, kT, vT]
                yT = big.tile([128, FC, S], OP_DT, tag="xyT")
                for hp in range(FC):
                    for m in range(3):
                        dst = qkvT[m]
                        acc0 = pj.tile([128, 512], F32, tag="pj")
                        acc1 = pj.tile([128, 512], F32, tag="pj")
                        accs = (acc0, acc1)
                        for ci in range(FC):
                            for h in range(2):
                                nc.tensor.matmul(
                                    accs[h][:, :NH],
                                    wsb[m][:, ci, hp * 128:(hp + 1) * 128],
                                    xT[:, ci, h * NH:(h + 1) * NH],
                                    start=(ci == 0), stop=(ci == FC - 1))
                        for h in range(2):
                            nc.vector.tensor_scalar_add(
                                dst[:, hp, h * NH:(h + 1) * NH],
                                in0=accs[h][:, :NH], scalar1=bsb[m][:, hp:hp + 1])

                    # V back to token-major, with an appended ones column
                    vn = vnp.tile([128, NS, 2, HD + 1], OP_DT, tag="vn")
                    nc.vector.tensor_copy(
                        vn[:ST, :, :, HD:HD + 1],
                        ones_r[:ST, 0:1].to_broadcast((ST, NS, 2, 1)))
                    for t0 in range(0, NS, 4):
                        nch = min(4, NS - t0)
                        ptv = ptr.tile([128, 512], OP_DT, tag="ptrv")
                        for t in range(nch):
                            nc.tensor.transpose(
                                ptv[:ST, t * 128:(t + 1) * 128],
                                vT[:, hp, (t0 + t) * ST:(t0 + t + 1) * ST],
                                ident_r[:, :])
                        nc.vector.tensor_copy(
                            vn[:ST, t0:t0 + nch, :, 0:HD],
                            ptv[:ST, :nch * 128].rearrange("p (t g d) -> p t g d", t=nch, g=2))
                    if b == 0 and hp == 0:
                        _dump("vn", vn[:])
                    for sh in range(2):
                        pp = ppool.tile([128, 2, NS, NH], OP_DT, tag="pp")
                        for g in range(2):
                            r0 = g * HD
                            for t5 in range(NS):
                                st_ps = ps.tile([128, 512], F32, tag="ps")
                                nc.tensor.matmul(
                                    st_ps[:ST, :NH],
                                    kT[r0:r0 + HD, hp, t5 * ST:(t5 + 1) * ST],
                                    qT[r0:r0 + HD, hp, sh * NH:(sh + 1) * NH],
                                    start=True, stop=True)
                                nc.scalar.activation(
                                    pp[:ST, g, t5, :], st_ps[:ST, :NH],
                                    func=mybir.ActivationFunctionType.Exp)
                            if b == 0 and hp == 0 and sh == 0 and g == 1:
                                _dump("pp", pp[:])
                            acc = py.tile([HD + 1, 512], F32, tag="py")
                            for t5 in range(NS):
                                nc.tensor.matmul(
                                    acc[:HD + 1, :NH],
                                    vn[:ST, t5, g, :],
                                    pp[:ST, g, t5, :],
                                    start=(t5 == 0), stop=(t5 == NS - 1))
                            srow = recp.tile([1, NH], F32, tag="srow")
                            nc.vector.tensor_copy(srow[:, :], acc[HD:HD + 1, :NH])
                            rec = recp.tile([1, NH], F32, tag="rec")
                            nc.vector.reciprocal_approx_fast(rec[:, :], srow[:, :])
                            rb = recp.tile([HD, NH], F32, tag="rb")
                            nc.gpsimd.partition_broadcast(rb[:, :], rec[0:1, :])
                            nc.vector.tensor_mul(
                                yT[r0:r0 + HD, hp, sh * NH:(sh + 1) * NH],
                                acc[:HD, :NH], rb[:, :])
                if b == 0:
                    _dump("qT", qT[:])
                    _dump("kT", kT[:])
                    _dump("vT", vT[:])
                    _dump("yT", yT[:])

                # ---- output projection ------------------------------------
                outT = big.tile([128, FC, S], F32, tag="outT")
                for co in range(FC):
                    acc0 = pj.tile([128, 512], F32, tag="pj")
                    acc1 = pj.tile([128, 512], F32, tag="pj")
                    accs = (acc0, acc1)
                    for ci in range(FC):
                        for h in range(2):
                            nc.tensor.matmul(
                                accs[h][:, :NH],
                                wsb[3][:, ci, co * 128:(co + 1) * 128],
                                yT[:, ci, h * NH:(h + 1) * NH],
                                start=(ci == 0), stop=(ci == FC - 1))
                    for h in range(2):
                        nc.vector.tensor_scalar_add(
                            outT[:, co, h * NH:(h + 1) * NH],
                            in0=accs[h][:, :NH], scalar1=bsb[3][:, co:co + 1])

                if b == 0:
                    _dump("outT", outT[:])

                # ---- back to token-major + store --------------------------
                for st in range(NS):
                    on = stage.tile([128, F], F32, tag="on")
                    for c0 in range(0, FC, 4):
                        nch = min(4, FC - c0)
                        pt = ptr.tile([128, 512], F32, tag="ptr")
                        for t in range(nch):
                            nc.tensor.transpose(
                                pt[:ST, t * 128:(t + 1) * 128],
                                outT[:, c0 + t, st * ST:(st + 1) * ST],
                                ident[:, :])
                        nc.vector.tensor_copy(
                            on[:ST, c0 * 128:(c0 + nch) * 128], pt[:ST, :nch * 128])
                    nc.sync.dma_start(out=outr[b, st * ST:(st + 1) * ST, :], in_=on[:ST, :])

    nc.finalize()
    return nc


_NC_CACHE = {}


def _get_nc():
    if "nc" not in _NC_CACHE:
        _NC_CACHE["nc"] = _build_nc()
    return _NC_CACHE["nc"]


# ------------------------------------------------------------- host wrapper
def _head_major_perm():
    perm = np.empty(F, dtype=np.int64)
    i = 0
    for h1 in range(H1):
        for h2 in range(H2):
            for h3 in range(H3):
                for x in range(XD):
                    for y in range(YD):
                        for z in range(ZD):
                            a = x * H1 + h1
                            bb = y * H2 + h2
                            cc = z * H3 + h3
                            perm[i] = a * 96 + bb * 12 + cc
                            i += 1
    return perm


def _prep_inputs(inputs):
    perm = _head_major_perm()
    scale = float(HD) ** -0.5

    def kron3(w1, w2, w3):
        return np.kron(w1, np.kron(w2, w3)).astype(np.float32)

    mats = {}
    mats["w0"] = np.ascontiguousarray(
        (kron3(inputs["Wq1"], inputs["Wq2"], inputs["Wq3"])[perm, :] * scale).T)
    mats["b0"] = np.ascontiguousarray(
        inputs["bq"].reshape(F)[perm] * scale).astype(np.float32)
    mats["w1"] = np.ascontiguousarray(
        kron3(inputs["Wk1"], inputs["Wk2"], inputs["Wk3"])[perm, :].T)
    mats["b1"] = np.ascontiguousarray(inputs["bk"].reshape(F)[perm]).astype(np.float32)
    mats["w2"] = np.ascontiguousarray(
        kron3(inputs["Wv1"], inputs["Wv2"], inputs["Wv3"])[perm, :].T)
    mats["b2"] = np.ascontiguousarray(inputs["bv"].reshape(F)[perm]).astype(np.float32)
    mats["w3"] = np.ascontiguousarray(
        kron3(inputs["Wo1"], inputs["Wo2"], inputs["Wo3"])[:, perm].T)
    mats["b3"] = np.ascontiguousarray(inputs["bo"].reshape(F)).astype(np.float32)
    return mats


def _make_in_maps(inputs):
    mats = _prep_inputs(inputs)
    if W_DT == mybir.dt.float16:
        for k in ("w0", "w1", "w2", "w3"):
            mats[k] = mats[k].astype(np.float16)
    x = np.ascontiguousarray(np.asarray(inputs["x"], dtype=np.float32).reshape(B, S, F))
    in_maps = []
    for c in range(N_CORES):
        m = {"x": np.ascontiguousarray(x[c * BPC:(c + 1) * BPC])}
        m.update(mats)
        in_maps.append(m)
    return in_maps


def kernel(**inputs) -> np.ndarray:
    nc = _get_nc()
    in_maps = _make_in_maps(inputs)
    res = run_bass_kernel_spmd(nc, in_maps, core_ids=list(range(N_CORES)))
    out = np.concatenate([res.results[c]["out"] for c in range(N_CORES)], axis=0)
    return out.reshape(B, P1, P2, 8, 8, 12).astype(np.float32)


def run_traced(inputs, **kw):
    """test.py helper: returns (output, BassKernelResults) with trace."""
    nc = _get_nc()
    in_maps = _make_in_maps(inputs)
    res = run_bass_kernel_spmd(nc, in_maps, core_ids=list(range(N_CORES)), **kw)
    out = np.concatenate([res.results[c]["out"] for c in range(N_CORES)], axis=0)
    return out.reshape(B, P1, P2, 8, 8, 12).astype(np.float32), res
